# revision 29
# baseline (speedup 1.0000x reference)
"""AttnBlock (GroupNorm -> QKV -> single-head attention over 4096 tokens -> proj
+ residual) on 8 Trainium2 NeuronCores, data-parallel over batch (B=8, one batch
element per core).

FP8 DoubleRow edition: every large matmul (QKV, scores, PV, proj) runs as
fp8e4m3 with perf_mode=DoubleRow -- the PE array virtualizes to 256 contraction
rows, so each matmul instruction retires 2x the MACs of a bf16 one at ~1x the
issue cost.  All operands live in "pair" tiles [128, 2, F] whose middle dim is
the contraction-pair index.

Numerics (tolerance is 2e-2 relative; measured ~0.48e-3 for the bf16 baseline):
  - Weights are scaled x16 on the host before the fp8 cast (their natural scale
    1/sqrt(C)~0.044 would land in fp8 subnormals); the 1/16 is folded into the
    PSUM->SBUF evacuation scale of q/k/v and cancels exactly for proj.
  - exp() gets a -2 logit shift (softmax-invariant) so fp8 never overflows
    (TRN fp8e4 -> Inf above 240), and the unnormalized PV output is stored
    /16 in fp8, with the 16 folded back via l (the ones-matmul sums the same
    shifted exps, so the ratio is exact).
  - pbp = proj_b + Wp @ v_b is computed on the host in f64 (the softmax
    division happens after the projection, so v's bias folds into a constant).
  - GroupNorm statistics, softmax sums, reciprocals and the residual add stay
    in f32/bf16 exactly as in the bf16 baseline.

Layout strategy (no on-chip transposes anywhere):
  - h, q, k in channel-major pair tiles [128, 2, 4096] (channel blocks (0,1)
    and (2,3)); v is produced directly transposed into key-pair tiles
    [128, 2, 512] by using h-tiles as the stationary matmul operand.
  - Scores are computed transposed, S_T[m, n] (keys m on partitions), per
    (key-pair, n-chunk); one fused exp() reads both PSUM banks of the pair
    straight into an fp8 pair tile (flash-style, never materializing the
    4096x4096 matrix).
  - The softmax denominator comes from a running element-wise sum of the exp
    tiles on the Vector engine plus one ones-matmul per query chunk (whose
    [128,512] output is l already broadcast across partitions).
"""

import sys
import types
from contextlib import ExitStack

import numpy as np

import concourse.bass as bass
import concourse.mybir as mybir
import concourse.tile as tile
from concourse.bass_utils import run_bass_kernel_spmd

dt = mybir.dt
AF = mybir.ActivationFunctionType
ALU = mybir.AluOpType
AX = mybir.AxisListType
PM = mybir.MatmulPerfMode

B = 8
C = 512
HW = 4096  # 64*64 pixels
P = 128
CT = C // P  # 4 channel tiles
CP = CT // 2  # 2 channel-pair tiles
NCH = HW // 512  # 8 n-chunks of 512 queries
MT = HW // P  # 32 key tiles of 128
MP = MT // 2  # 16 key-pair tiles of 256
NPIX = 16 * HW  # elements per group (16 channels x 4096 pixels)
EPS = 1e-6
SCALE = float(C) ** -0.5
WSC = 16.0  # host-side weight scale (power of 2), folded back on chip
ESH = -2.0  # softmax logit shift (softmax-invariant, keeps exp in fp8 range)

_CACHE = {}


def _legalize_waits(nc, cap=1):
    """This walrus build rejects instructions with more than a couple of
    semaphore waits in sync_info (Tile packs all end-of-kernel waits into one
    Drain).  Split excess waits into single-wait NoOps in front."""
    for f in nc.m.functions:
        for b in f.blocks:
            insts = b.instructions
            i = 0
            while i < len(insts):
                ins = insts[i]
                si = ins.sync_info
                if si is not None and len(si.on_wait) > cap:
                    waits = list(si.on_wait)
                    for j, w in enumerate(waits[:-cap]):
                        nop = mybir.InstNoOp(
                            name=f"wsplit_{ins.name}_{j}", ins=[], outs=[]
                        )
                        nop.engine = ins.engine
                        nop.sync_info = mybir.SyncInfo(on_wait=[w], on_update=[])
                        insts.insert(i, nop)
                        i += 1
                    ins.sync_info = mybir.SyncInfo(
                        on_wait=waits[-cap:], on_update=list(si.on_update)
                    )
                i += 1


def _build():
    nc = bass.Bass(
        "TRN2", target_bir_lowering=False, debug=False, num_swdge_queues=4
    )
    x_d = nc.dram_tensor("x", [C, HW], dt.float32, kind="ExternalInput").ap()
    xbf_d = nc.dram_tensor("x_bf", [C, HW], dt.float8e4, kind="ExternalInput").ap()
    y_d = nc.dram_tensor("y", [C, HW], dt.float32, kind="ExternalOutput").ap()
    w_d = {
        n: nc.dram_tensor(n, [C, C], dt.float8e4, kind="ExternalInput").ap()
        for n in ("wq_t", "wk_t", "wv_t", "wp_t")
    }
    # packed per-channel params: [C, 5] = qb,kb,pbp,gnw,gnb
    bpack_d = nc.dram_tensor("bpack", [C, 5], dt.float32, kind="ExternalInput").ap()
    gmat_d = nc.dram_tensor("gmat", [P, 8], dt.float32, kind="ExternalInput").ap()
    gmt_d = nc.dram_tensor("gmat_t", [8, P], dt.float32, kind="ExternalInput").ap()

    with tile.TileContext(nc) as tc, ExitStack() as ctx:
        const = ctx.enter_context(tc.tile_pool(name="const", bufs=1))
        qpool = ctx.enter_context(tc.tile_pool(name="qpool", bufs=1))
        kpool = ctx.enter_context(tc.tile_pool(name="kpool", bufs=1))
        vtpool = ctx.enter_context(tc.tile_pool(name="vtpool", bufs=1))

        # x (staged bf16 copy) is the head-latency critical input: issue its
        # DMAs first, split across all DMA queues (2 HWDGE + 4 SWDGE). GpSimd
        # gets few issues per tile -- its queue also runs the GN square for
        # one quarter of each tile, which must not sit behind descriptor
        # generation (~0.65us each) for later tiles.
        x_eng_01 = [nc.sync, nc.scalar, nc.sync, nc.gpsimd,
                    nc.gpsimd, nc.gpsimd, nc.sync, nc.scalar]
        x_eng_23 = [nc.sync, nc.sync, nc.gpsimd, nc.gpsimd,
                    nc.gpsimd, nc.sync, nc.sync, nc.scalar]

        def load_x(xt, t):
            r = slice(t * P, (t + 1) * P)
            engs = x_eng_01 if t < 2 else x_eng_23
            for s in range(8):
                cs = slice(s * 512, (s + 1) * 512)
                engs[s].dma_start(xt[:, cs], xbf_d[r, cs])

        phase1 = tc.tile_pool(name="phase1", bufs=1)
        ph1 = phase1.__enter__()
        xpool_cm = tc.tile_pool(name="xload", bufs=4)
        xpool = xpool_cm.__enter__()
        x_tiles = []
        for t in range(CT):
            xt = xpool.tile([P, HW], dt.float8e4, tag="xt", name=f"x{t}")
            load_x(xt, t)
            x_tiles.append(xt)

        # constants / weights go to the 4 SWDGE queues via gpsimd (idle engine)
        bpk = const.tile([P, CT, 5], dt.float32, tag="bpk")
        nc.sync.dma_start(bpk[:], bpack_d.rearrange("(t p) k -> p t k", p=P))
        gmat = const.tile([P, 8], dt.float32, tag="gmat")
        nc.sync.dma_start(gmat[:], gmat_d[:, :])
        gmt = const.tile([8, P], dt.float32, tag="gmt")
        nc.sync.dma_start(gmt[:], gmt_d[:, :])
        ones_col = const.tile([P, P], dt.bfloat16, tag="ones_col")
        nc.vector.memset(ones_col[:], 1.0)
        nbias = const.tile([P, 1], dt.float32, tag="nbias")
        nc.vector.memset(nbias[:], ESH)
        ones_pair = const.tile([P, 2, P], dt.float8e4, tag="ones_pair")
        nc.vector.memset(ones_pair[:], 1.0)

        BIDX = {"qb": 0, "kb": 1, "pbp": 2, "gnw": 3, "gnb": 4}
        bias = {n: [bpk[:, t, k : k + 1] for t in range(CT)] for n, k in BIDX.items()}

        # ---------------- GroupNorm -> h (fp8 pair tiles [128, 2, HW]) -------
        h2 = [
            ph1.tile([P, 2, HW], dt.float8e4, tag=f"h{c}", name=f"h{c}")
            for c in range(CP)
        ]
        with (
            tc.tile_pool(name="gnscr", bufs=2) as scr,
            tc.tile_pool(name="gnstat", bufs=2) as stat,
            tc.tile_pool(name="gnps", bufs=2, space="PSUM") as gnps,
        ):
            for t in range(CT):
                xt = x_tiles[t]
                stats = stat.tile([P, 2], dt.float32, tag="stats")
                # engine split per tile (~5-6us each, pipelined across tiles):
                #   ACT: squares jc0..2 (Square+accum) + x-sum half B
                #   DVE: x-sum half A + the reduce of GpSimd's square
                #   GP : square jc3 (multiply only)
                sqacc = stat.tile([P, 4], dt.float32, tag="sqacc")
                for jc in range(3):
                    sq = scr.tile([P, 1024], dt.float32, tag="sq")
                    nc.scalar.activation(
                        sq[:],
                        xt[:, jc * 1024 : (jc + 1) * 1024],
                        AF.Square,
                        accum_out=sqacc[:, jc : jc + 1],
                    )
                sqg = scr.tile([P, 1024], dt.float32, tag="sqg")
                nc.gpsimd.tensor_tensor(sqg[:], xt[:, 3072:4096], xt[:, 3072:4096],
                                        op=ALU.mult)
                nc.vector.tensor_reduce(sqacc[:, 3:4], sqg[:], axis=AX.X, op=ALU.add)
                s1acc = stat.tile([P, 2], dt.float32, tag="s1acc")
                nc.vector.tensor_reduce(
                    s1acc[:, 0:1], xt[:, 0:2048], axis=AX.X, op=ALU.add
                )
                sxb = scr.tile([P, 2048], dt.float32, tag="sxb")
                nc.scalar.activation(
                    sxb[:], xt[:, 2048:4096], AF.Identity,
                    accum_out=s1acc[:, 1:2],
                )
                nc.vector.tensor_reduce(stats[:, 0:1], s1acc[:], axis=AX.X, op=ALU.add)
                nc.vector.tensor_reduce(stats[:, 1:2], sqacc[:], axis=AX.X, op=ALU.add)
                gps = gnps.tile([8, 2], dt.float32, tag="gps")
                nc.tensor.matmul(gps[:], gmat[:], stats[:], start=True, stop=True)
                gsb = stat.tile([8, 2], dt.float32, tag="gsb")
                nc.vector.tensor_copy(gsb[:], gps[:])
                cps = gnps.tile([P, 2], dt.float32, tag="cps")
                nc.tensor.matmul(cps[:], gmt[:], gsb[:], start=True, stop=True)
                cst = stat.tile([P, 2], dt.float32, tag="cst")
                nc.vector.tensor_copy(cst[:], cps[:])
                mean = stat.tile([P, 1], dt.float32, tag="mean")
                nc.vector.tensor_scalar_mul(mean[:], cst[:, 0:1], 1.0 / NPIX)
                msq = stat.tile([P, 1], dt.float32, tag="msq")
                nc.scalar.square(msq[:], mean[:])
                ex2 = stat.tile([P, 1], dt.float32, tag="ex2")
                nc.vector.tensor_scalar(
                    ex2[:], cst[:, 1:2], 1.0 / NPIX, EPS, op0=ALU.mult, op1=ALU.add
                )
                varp = stat.tile([P, 1], dt.float32, tag="varp")
                nc.vector.tensor_sub(varp[:], ex2[:], msq[:])
                rinv = stat.tile([P, 1], dt.float32, tag="rinv")
                nc.vector.reciprocal(rinv[:], varp[:])
                rstd = stat.tile([P, 1], dt.float32, tag="rstd")
                nc.scalar.sqrt(rstd[:], rinv[:])
                s_t = stat.tile([P, 1], dt.float32, tag="s_t")
                nc.vector.tensor_mul(s_t[:], rstd[:], bias["gnw"][t][:])
                ms = stat.tile([P, 1], dt.float32, tag="ms")
                nc.vector.tensor_mul(ms[:], mean[:], s_t[:])
                t_t = stat.tile([P, 1], dt.float32, tag="t_t")
                nc.vector.scalar_tensor_tensor(
                    t_t[:], ms[:], -1.0, bias["gnb"][t][:], op0=ALU.mult, op1=ALU.add
                )
                # h = x*s + t, split ACT/DVE/GpSimd
                dst = h2[t // 2][:, t % 2, :]
                nc.vector.tensor_scalar(
                    dst[0:P, 0:1536], xt[:, 0:1536],
                    s_t[:], t_t[:], op0=ALU.mult, op1=ALU.add,
                )
                nc.gpsimd.tensor_scalar(
                    dst[0:P, 1536:2560], xt[:, 1536:2560],
                    s_t[:], t_t[:], op0=ALU.mult, op1=ALU.add,
                )
                nc.scalar.activation(
                    dst[0:P, 2560:4096], xt[:, 2560:4096], AF.Identity,
                    bias=t_t[:], scale=s_t[:],
                )
        xpool_cm.__exit__(None, None, None)

        # weights (fp8, x16 from the host; loaded after GN emission so their
        # DMAs don't sit ahead of the GN reductions in queue order). wq/wk
        # gate the QKV start: put them on the SWDGE queues, which finish
        # their share of x earliest; wv/wp (needed later) go to the HW queues.
        w_engine = {"wq_t": nc.sync, "wk_t": nc.sync, "wv_t": nc.sync,
                    "wp_t": nc.sync}
        w2 = {}
        for n in ("wq_t", "wk_t", "wv_t", "wp_t"):
            pool = const if n == "wp_t" else ph1
            tiles = []
            for cc in range(CP):
                wb = pool.tile(
                    [P, 2, C], dt.float8e4, tag=f"{n}f8{cc}", name=f"{n}f8{cc}"
                )
                for h in range(2):
                    t = 2 * cc + h
                    w_engine[n].dma_start(wb[:, h, :], w_d[n][t * P : (t + 1) * P, :])
                tiles.append(wb)
            w2[n] = tiles
        wp2 = w2["wp_t"]

        # ------- q, k (fp8 pair tiles [128,2,HW]) and v_T ([128,2,C] x16) ----
        q2 = [
            qpool.tile([P, 2, HW], dt.float8e4, tag=f"q{c}", name=f"q{c}")
            for c in range(CP)
        ]
        k2 = [
            kpool.tile([P, 2, HW], dt.float8e4, tag=f"k{c}", name=f"k{c}")
            for c in range(CP)
        ]
        vt2 = [
            vtpool.tile([P, 2, C], dt.float8e4, tag=f"vt{m}", name=f"vt{m}")
            for m in range(MP)
        ]
        with tc.tile_pool(name="qkps", bufs=8, space="PSUM") as qkps:
            for wn, dst2, bn in (("wq_t", q2, "qb"), ("wk_t", k2, "kb")):
                for o in range(CT):
                    for j in range(NCH):
                        ps = qkps.tile([P, 512], dt.float32, tag="qkps")
                        for cc in range(CP):
                            nc.tensor.matmul(
                                ps[:],
                                w2[wn][cc][:, :, o * P : (o + 1) * P],
                                h2[cc][:, :, j * 512 : (j + 1) * 512],
                                start=(cc == 0),
                                stop=(cc == CP - 1),
                                perf_mode=PM.DoubleRow,
                            )
                        # evacuate (x 1/WSC, + bias), alternating DVE/ACT
                        d = dst2[o // 2][:, o % 2, j * 512 : (j + 1) * 512]
                        if j % 2 == 0:
                            nc.vector.tensor_scalar(
                                d, ps[:], 1.0 / WSC, bias[bn][o][:],
                                op0=ALU.mult, op1=ALU.add,
                            )
                        else:
                            nc.scalar.activation(
                                d, ps[:], AF.Identity,
                                bias=bias[bn][o][:], scale=1.0 / WSC,
                            )
            for m in range(MT):
                ps = qkps.tile([P, 512], dt.float32, tag="qkps")
                for cc in range(CP):
                    nc.tensor.matmul(
                        ps[:],
                        h2[cc][:, :, m * P : (m + 1) * P],
                        w2["wv_t"][cc][:],
                        start=(cc == 0),
                        stop=(cc == CP - 1),
                        perf_mode=PM.DoubleRow,
                    )
                d = vt2[m // 2][:, m % 2, :]
                if m % 2 == 0:
                    nc.vector.tensor_scalar_mul(d, ps[:], 1.0 / WSC)
                else:
                    nc.scalar.activation(d, ps[:], AF.Identity, scale=1.0 / WSC)
        phase1.__exit__(None, None, None)  # frees h + wq/wk/wv fp8 SBUF

        # ---------------- attention + proj, per n-chunk of 512 queries --------
        with (
            tc.tile_pool(name="spool", bufs=2, space="PSUM") as spool,
            tc.tile_pool(name="pvps", bufs=1, space="PSUM") as pvps,
            tc.tile_pool(name="ptpool", bufs=8) as ptpool,
            tc.tile_pool(name="ptacc", bufs=3) as ptaccp,
            tc.tile_pool(name="misc", bufs=2) as misc,
            tc.tile_pool(name="xres", bufs=4) as xres,
            tc.tile_pool(name="ystage", bufs=4) as ystage,
        ):
            def emit_proj(j, hu, linv_bc, op_list=(0, 1)):
                # y = P(hu) * (1/l) + pbp + x  -- division after the projection
                # (wp is x16 and hu is /16 in fp8, so the scales cancel here)
                for op in op_list:
                    pt_ps = spool.tile(
                        [P, 2, 512], dt.float32, tag="sp", name=f"pps{j}_{op}"
                    )
                    for oh in range(2):
                        o = 2 * op + oh
                        pps = pt_ps[:, oh, :]
                        for cc in range(CP):
                            nc.tensor.matmul(
                                pps,
                                wp2[cc][:, :, o * P : (o + 1) * P],
                                hu[cc][:],
                                start=(cc == 0),
                                stop=(cc == CP - 1),
                                perf_mode=PM.DoubleRow,
                            )
                        xr = xres.tile(
                            [P, 512], dt.float32, tag="xr", name=f"xr{j}_{o}"
                        )
                        nc.sync.dma_start(
                            xr[:], x_d[o * P : (o + 1) * P, j * 512 : (j + 1) * 512]
                        )
                        ppc = ystage.tile(
                            [P, 512], dt.float32, tag="ppc", name=f"ppc{j}_{o}"
                        )
                        nc.scalar.copy(ppc[:], pps)
                        pn = ystage.tile(
                            [P, 512], dt.float32, tag="pn", name=f"pn{j}_{o}"
                        )
                        yst = ystage.tile(
                            [P, 512], dt.float32, tag="yst", name=f"y{j}_{o}"
                        )
                        halves = (slice(0, 256), slice(256, 512)) if j == NCH - 1 \
                            else (slice(0, 512),)
                        for hs in halves:
                            nc.vector.tensor_mul(pn[:, hs], ppc[:, hs],
                                                 linv_bc[:, hs])
                            nc.vector.scalar_tensor_tensor(
                                yst[:, hs], pn[:, hs], bias["pbp"][o][:], xr[:, hs],
                                op0=ALU.add, op1=ALU.add,
                            )
                        nc.sync.dma_start(
                            y_d[o * P : (o + 1) * P, j * 512 : (j + 1) * 512], yst[:]
                        )

            def emit_norm_a(j, accs, pt_last, split=False):
                # ones-matmuls merge both partial exp sums (pairs 0..14) plus
                # the final exp pair straight from fp8 (so the norm does not
                # chase the accumulator engines at the chunk seam): out[m, n]
                # = l[n] on every row -- broadcast for free; then reciprocal.
                lt = spool.tile([P, 2, 512], dt.float32, tag="sp", name=f"lf{j}")
                nc.tensor.matmul(lt[:, 0, :], ones_col[:], accs[0][:],
                                 start=True, stop=False)
                nc.tensor.matmul(lt[:, 0, :], ones_col[:], accs[1][:],
                                 start=False, stop=False)
                nc.tensor.matmul(lt[:, 0, :], ones_pair[:], pt_last[:],
                                 start=False, stop=True, perf_mode=PM.DoubleRow)
                # copy l out of PSUM first: the slow reciprocal must not pin
                # the PSUM ring slot (the next scores alloc waits on it)
                lcp = misc.tile([P, 512], dt.float32, tag="lcp", name=f"lcp{j}")
                nc.scalar.copy(lcp[:], lt[:, 0, :])
                linv_bc = misc.tile([P, 512], dt.float32, tag="linvbc", name=f"lbc{j}")
                if split:
                    # tail: halve the serial reciprocal so the first proj pn
                    # can start ~1.6us earlier
                    nc.vector.reciprocal(linv_bc[:, 0:256], lcp[:, 0:256])
                    nc.vector.reciprocal(linv_bc[:, 256:512], lcp[:, 256:512])
                else:
                    nc.vector.reciprocal(linv_bc[:], lcp[:])
                return linv_bc

            # flat software pipeline over all (chunk, key-pair) steps: the PV
            # matmuls trail the score/exp stream by a global lag of 2 pairs,
            # flowing across chunk boundaries so the PE never sees a seam.
            PVLAG = 4
            state = {}   # j -> dict(pv_ps, accs, pts)
            hu_by = {}   # j -> hu pair tiles
            linv_by = {}

            def emit_scores(j, ii):
                st = state[j]
                s_ps = spool.tile([P, 2, 512], dt.float32, tag="sp", name=f"s{j}_{ii}")
                for h in range(2):
                    i = 2 * ii + h
                    for cc in range(CP):
                        nc.tensor.matmul(
                            s_ps[:, h, :],
                            k2[cc][:, :, i * P : (i + 1) * P],
                            q2[cc][:, :, j * 512 : (j + 1) * 512],
                            start=(cc == 0),
                            stop=(cc == CP - 1),
                            perf_mode=PM.DoubleRow,
                        )
                pt = ptpool.tile([P, 2, 512], dt.float8e4, tag="pt",
                                 name=f"pt{j}_{ii}")
                # fused exp over both PSUM banks, with the -2 logit shift
                nc.scalar.activation(pt[:], s_ps[:], AF.Exp, bias=nbias[:],
                                     scale=SCALE)
                st["pts"][ii] = pt
                accA, accB = st["accs"]
                if ii == 0:
                    nc.vector.tensor_copy(accA[:], pt[:, 0, :])
                    nc.gpsimd.tensor_copy(accB[:], pt[:, 1, :])
                elif ii < MP - 1:  # last pair is summed by the norm matmul
                    nc.vector.tensor_add(accA[:], accA[:], pt[:, 0, :])
                    nc.gpsimd.tensor_add(accB[:], accB[:], pt[:, 1, :])

            def emit_pv(j, ii):
                st = state[j]
                for c in range(CT):
                    nc.tensor.matmul(
                        st["pv_ps"][c][:],
                        vt2[ii][:, :, c * P : (c + 1) * P],
                        st["pts"][ii][:],
                        start=(ii == 0),
                        stop=(ii == MP - 1),
                        perf_mode=PM.DoubleRow,
                    )

            def emit_hu_evac(j):
                # unnormalized PV (/16) to fp8 SBUF frees the banks for the
                # next chunk; split DVE/ACT so neither stream hiccups
                hu = [
                    misc.tile([P, 2, 512], dt.float8e4, tag=f"hu{cc}",
                              name=f"hu{j}_{cc}")
                    for cc in range(CP)
                ]
                pv_ps = state[j]["pv_ps"]
                for c in range(CT):
                    d = hu[c // 2][:, c % 2, :]
                    if c < 2 and j < NCH - 1:
                        nc.vector.tensor_scalar_mul(d, pv_ps[c][:], 1.0 / WSC)
                    else:
                        nc.scalar.activation(d, pv_ps[c][:], AF.Identity,
                                             scale=1.0 / WSC)
                hu_by[j] = hu

            NSTEP = NCH * MP
            pv_next = 0
            for g in range(NSTEP + 2):
                j, ii = divmod(g, MP)
                if ii == 0 and j < NCH:
                    state[j] = {
                        "pv_ps": [
                            pvps.tile([P, 512], dt.float32, tag=f"pv{c}",
                                      name=f"pv{j}_{c}")
                            for c in range(CT)
                        ],
                        # two independent running exp sums (DVE + GpSimd),
                        # merged by the two-step ones-matmul in emit_norm_a
                        "accs": (
                            ptaccp.tile([P, 512], dt.bfloat16, tag="accA",
                                        name=f"accA{j}"),
                            ptaccp.tile([P, 512], dt.bfloat16, tag="accB",
                                        name=f"accB{j}"),
                        ),
                        "pts": [None] * MP,
                    }
                if g < NSTEP:
                    emit_scores(j, ii)
                # PV trails by PVLAG pairs; in the last two score slots it
                # catches up (2 steps each) so the end-of-kernel drain halves
                n_emit = 0 if g < PVLAG else (2 if g >= NSTEP - 2 else 1)
                for _ in range(n_emit):
                    if pv_next >= NSTEP:
                        break
                    jp, iip = divmod(pv_next, MP)
                    pv_next += 1
                    emit_pv(jp, iip)
                    if iip == MP - 1:
                        emit_hu_evac(jp)
                # per-chunk norm/proj of the previous chunk, off the seam;
                # the last chunk's norm fires immediately after its final exp
                # so the tail reciprocal overlaps the PV drain
                if ii == 1 and 1 <= j < NCH:
                    linv_by[j - 1] = emit_norm_a(
                        j - 1, state[j - 1]["accs"], state[j - 1]["pts"][MP - 1]
                    )
                if ii == 4 and 1 <= j < NCH:
                    emit_proj(j - 1, hu_by[j - 1], linv_by[j - 1], op_list=(0,))
                if ii == 6 and 1 <= j < NCH:
                    emit_proj(j - 1, hu_by[j - 1], linv_by[j - 1], op_list=(1,))
                if j == NCH - 1 and ii == MP - 1:
                    linv_by[j] = emit_norm_a(j, state[j]["accs"],
                                             state[j]["pts"][MP - 1], split=True)

            emit_proj(NCH - 1, hu_by[NCH - 1], linv_by[NCH - 1])

    _legalize_waits(nc)
    return nc


def _get_nc():
    if "nc" not in _CACHE:
        _CACHE["nc"] = _build()
    return _CACHE["nc"]


def _in_maps(x, gn_w, gn_b, q_w, q_b, k_w, k_b, v_w, v_b, proj_w, proj_b):
    x = np.ascontiguousarray(np.asarray(x, dtype=np.float32))
    assert x.shape == (B, C, 64, 64)
    f32 = np.float32
    gmat = np.zeros((P, 8), f32)
    gmat[np.arange(P), np.arange(P) // 16] = 1.0
    # pbp = proj_b + Wp @ v_b (division-after-projection folds v's bias into a
    # constant output bias); computed on host in f64.
    pbp = (
        np.asarray(proj_b, np.float64)
        + np.asarray(proj_w, np.float64) @ np.asarray(v_b, np.float64)
    ).astype(f32)
    bpack = np.stack(
        [
            np.asarray(a, f32).reshape(C)
            for a in (q_b, k_b, pbp, gn_w, gn_b)
        ],
        axis=1,
    )  # [C, 5]
    import ml_dtypes

    bf16 = ml_dtypes.bfloat16
    f8 = ml_dtypes.float8_e4m3

    def wprep(w):
        return np.ascontiguousarray(
            (np.asarray(w, f32).T * np.float32(WSC)).astype(f8)
        )

    shared = {
        "wq_t": wprep(q_w),
        "wk_t": wprep(k_w),
        "wv_t": wprep(v_w),
        "wp_t": wprep(proj_w),
        "bpack": np.ascontiguousarray(bpack),
        "gmat": gmat,
        "gmat_t": np.ascontiguousarray(gmat.T),
    }

    return [
        dict(
            shared,
            x=x[b].reshape(C, HW),
            x_bf=x[b].reshape(C, HW).astype(f8),
        )
        for b in range(B)
    ]


def kernel(x, gn_w, gn_b, q_w, q_b, k_w, k_b, v_w, v_b, proj_w, proj_b):
    in_maps = _in_maps(x, gn_w, gn_b, q_w, q_b, k_w, k_b, v_w, v_b, proj_w, proj_b)
    nc = _get_nc()
    res = run_bass_kernel_spmd(nc, in_maps, core_ids=list(range(B)))
    out = np.stack([res.results[b]["y"].reshape(C, 64, 64) for b in range(B)])
    return out.astype(np.float32)


def run_traced(x, gn_w, gn_b, q_w, q_b, k_w, k_b, v_w, v_b, proj_w, proj_b):
    """Like kernel() but with NTFF profiling; returns (out, results)."""
    _install_ntff_hook()
    in_maps = _in_maps(x, gn_w, gn_b, q_w, q_b, k_w, k_b, v_w, v_b, proj_w, proj_b)
    nc = _get_nc()
    res = run_bass_kernel_spmd(nc, in_maps, core_ids=list(range(B)), trace=True)
    out = np.stack([res.results[b]["y"].reshape(C, 64, 64) for b in range(B)])
    return out.astype(np.float32), res


def _install_ntff_hook():
    if "antenv.axon_hooks" in sys.modules:
        return
    sys.path.insert(0, "/root/.axon_site")
    try:
        from trn_agent_boot.trn_boot import _ntff_profile_via_ctypes

        hook = _ntff_profile_via_ctypes("/opt/axon/libaxon_pjrt.so")
    except Exception:
        hook = None
    mod = types.ModuleType("antenv.axon_hooks")
    mod.get_axon_ntff_profile_hook = lambda: hook
    sys.modules["antenv.axon_hooks"] = mod


# revision 30
# speedup vs baseline: 1.1959x; 1.1959x over previous
"""AttnBlock (GroupNorm -> QKV -> single-head attention over 4096 tokens -> proj
+ residual) on 8 Trainium2 NeuronCores, data-parallel over batch (B=8, one batch
element per core).

FP8 DoubleRow edition: every large matmul (QKV, scores, PV, proj) runs as
fp8e4m3 with perf_mode=DoubleRow -- the PE array virtualizes to 256 contraction
rows, so each matmul instruction retires 2x the MACs of a bf16 one at ~1x the
issue cost.  All operands live in "pair" tiles [128, 2, F] whose middle dim is
the contraction-pair index.

Numerics (tolerance is 2e-2 relative; measured ~0.48e-3 for the bf16 baseline):
  - Weights are scaled x16 on the host before the fp8 cast (their natural scale
    1/sqrt(C)~0.044 would land in fp8 subnormals); the 1/16 is folded into the
    PSUM->SBUF evacuation scale of q/k/v and cancels exactly for proj.
  - exp() gets a -2 logit shift (softmax-invariant) so fp8 never overflows
    (TRN fp8e4 -> Inf above 240), and the unnormalized PV output is stored
    /16 in fp8, with the 16 folded back via l (the ones-matmul sums the same
    shifted exps, so the ratio is exact).
  - pbp = proj_b + Wp @ v_b is computed on the host in f64 (the softmax
    division happens after the projection, so v's bias folds into a constant).
  - GroupNorm statistics, softmax sums, reciprocals and the residual add stay
    in f32/bf16 exactly as in the bf16 baseline.

Layout strategy (no on-chip transposes anywhere):
  - h, q, k in channel-major pair tiles [128, 2, 4096] (channel blocks (0,1)
    and (2,3)); v is produced directly transposed into key-pair tiles
    [128, 2, 512] by using h-tiles as the stationary matmul operand.
  - Scores are computed transposed, S_T[m, n] (keys m on partitions), per
    (key-pair, n-chunk); one fused exp() reads both PSUM banks of the pair
    straight into an fp8 pair tile (flash-style, never materializing the
    4096x4096 matrix).
  - The softmax denominator comes from a running element-wise sum of the exp
    tiles on the Vector engine plus one ones-matmul per query chunk (whose
    [128,512] output is l already broadcast across partitions).
"""

import sys
import types
from contextlib import ExitStack

import numpy as np

import concourse.bass as bass
import concourse.mybir as mybir
import concourse.tile as tile
from concourse.bass_utils import run_bass_kernel_spmd

dt = mybir.dt
AF = mybir.ActivationFunctionType
ALU = mybir.AluOpType
AX = mybir.AxisListType
PM = mybir.MatmulPerfMode

B = 8
C = 512
HW = 4096  # 64*64 pixels
P = 128
CT = C // P  # 4 channel tiles
CP = CT // 2  # 2 channel-pair tiles
NCH = HW // 512  # 8 n-chunks of 512 queries
MT = HW // P  # 32 key tiles of 128
MP = MT // 2  # 16 key-pair tiles of 256
NPIX = 16 * HW  # elements per group (16 channels x 4096 pixels)
EPS = 1e-6
SCALE = float(C) ** -0.5
WSC = 16.0  # host-side weight scale (power of 2), folded back on chip
ESH = -2.0  # softmax logit shift (softmax-invariant, keeps exp in fp8 range)

_CACHE = {}


def _legalize_waits(nc, cap=1):
    """This walrus build rejects instructions with more than a couple of
    semaphore waits in sync_info (Tile packs all end-of-kernel waits into one
    Drain).  Split excess waits into single-wait NoOps in front."""
    for f in nc.m.functions:
        for b in f.blocks:
            insts = b.instructions
            i = 0
            while i < len(insts):
                ins = insts[i]
                si = ins.sync_info
                if si is not None and len(si.on_wait) > cap:
                    waits = list(si.on_wait)
                    for j, w in enumerate(waits[:-cap]):
                        nop = mybir.InstNoOp(
                            name=f"wsplit_{ins.name}_{j}", ins=[], outs=[]
                        )
                        nop.engine = ins.engine
                        nop.sync_info = mybir.SyncInfo(on_wait=[w], on_update=[])
                        insts.insert(i, nop)
                        i += 1
                    ins.sync_info = mybir.SyncInfo(
                        on_wait=waits[-cap:], on_update=list(si.on_update)
                    )
                i += 1


def _build():
    nc = bass.Bass(
        "TRN2", target_bir_lowering=False, debug=False, num_swdge_queues=4
    )
    x_d = nc.dram_tensor("x", [C, HW], dt.float32, kind="ExternalInput").ap()
    xbf_d = nc.dram_tensor("x_bf", [C, HW], dt.float8e4, kind="ExternalInput").ap()
    y_d = nc.dram_tensor("y", [C, HW], dt.float32, kind="ExternalOutput").ap()
    w_d = {
        n: nc.dram_tensor(n, [C, C], dt.float8e4, kind="ExternalInput").ap()
        for n in ("wq_t", "wk_t", "wv_t", "wp_t")
    }
    # packed per-channel params: [C, 5] = qb,kb,pbp,gnw,gnb
    bpack_d = nc.dram_tensor("bpack", [C, 5], dt.float32, kind="ExternalInput").ap()
    gmat_d = nc.dram_tensor("gmat", [P, 8], dt.float32, kind="ExternalInput").ap()
    gmt_d = nc.dram_tensor("gmat_t", [8, P], dt.float32, kind="ExternalInput").ap()

    with tile.TileContext(nc) as tc, ExitStack() as ctx:
        const = ctx.enter_context(tc.tile_pool(name="const", bufs=1))
        qpool = ctx.enter_context(tc.tile_pool(name="qpool", bufs=1))
        kpool = ctx.enter_context(tc.tile_pool(name="kpool", bufs=1))
        vtpool = ctx.enter_context(tc.tile_pool(name="vtpool", bufs=1))

        # x (staged bf16 copy) is the head-latency critical input: issue its
        # DMAs first, split across all DMA queues (2 HWDGE + 4 SWDGE). GpSimd
        # gets few issues per tile -- its queue also runs the GN square for
        # one quarter of each tile, which must not sit behind descriptor
        # generation (~0.65us each) for later tiles.
        x_eng_01 = [nc.sync, nc.scalar, nc.sync, nc.gpsimd,
                    nc.gpsimd, nc.gpsimd, nc.sync, nc.scalar]
        x_eng_23 = [nc.sync, nc.sync, nc.gpsimd, nc.gpsimd,
                    nc.gpsimd, nc.sync, nc.sync, nc.scalar]

        def load_x(xt, t):
            r = slice(t * P, (t + 1) * P)
            engs = x_eng_01 if t < 2 else x_eng_23
            for s in range(8):
                cs = slice(s * 512, (s + 1) * 512)
                engs[s].dma_start(xt[:, cs], xbf_d[r, cs])

        phase1 = tc.tile_pool(name="phase1", bufs=1)
        ph1 = phase1.__enter__()
        xpool_cm = tc.tile_pool(name="xload", bufs=4)
        xpool = xpool_cm.__enter__()
        x_tiles = []
        for t in range(CT):
            xt = xpool.tile([P, HW], dt.float8e4, tag="xt", name=f"x{t}")
            load_x(xt, t)
            x_tiles.append(xt)

        # constants / weights go to the 4 SWDGE queues via gpsimd (idle engine)
        bpk = const.tile([P, CT, 5], dt.float32, tag="bpk")
        nc.sync.dma_start(bpk[:], bpack_d.rearrange("(t p) k -> p t k", p=P))
        gmat = const.tile([P, 8], dt.float32, tag="gmat")
        nc.sync.dma_start(gmat[:], gmat_d[:, :])
        gmt = const.tile([8, P], dt.float32, tag="gmt")
        nc.sync.dma_start(gmt[:], gmt_d[:, :])
        ones_col = const.tile([P, P], dt.bfloat16, tag="ones_col")
        nc.vector.memset(ones_col[:], 1.0)
        nbias = const.tile([P, 1], dt.float32, tag="nbias")
        nc.vector.memset(nbias[:], ESH)
        ones_pair = const.tile([P, 2, P], dt.float8e4, tag="ones_pair")
        nc.vector.memset(ones_pair[:], 1.0)

        BIDX = {"qb": 0, "kb": 1, "pbp": 2, "gnw": 3, "gnb": 4}
        bias = {n: [bpk[:, t, k : k + 1] for t in range(CT)] for n, k in BIDX.items()}

        # ---------------- GroupNorm -> h (fp8 pair tiles [128, 2, HW]) -------
        h2 = [
            ph1.tile([P, 2, HW], dt.float8e4, tag=f"h{c}", name=f"h{c}")
            for c in range(CP)
        ]
        with (
            tc.tile_pool(name="gnscr", bufs=2) as scr,
            tc.tile_pool(name="gnstat", bufs=2) as stat,
            tc.tile_pool(name="gnps", bufs=2, space="PSUM") as gnps,
        ):
            for t in range(CT):
                xt = x_tiles[t]
                stats = stat.tile([P, 2], dt.float32, tag="stats")
                # engine split per tile (~5-6us each, pipelined across tiles):
                #   ACT: squares jc0..2 (Square+accum) + x-sum half B
                #   DVE: x-sum half A + the reduce of GpSimd's square
                #   GP : square jc3 (multiply only)
                sqacc = stat.tile([P, 4], dt.float32, tag="sqacc")
                for jc in range(3):
                    sq = scr.tile([P, 1024], dt.float32, tag="sq")
                    nc.scalar.activation(
                        sq[:],
                        xt[:, jc * 1024 : (jc + 1) * 1024],
                        AF.Square,
                        accum_out=sqacc[:, jc : jc + 1],
                    )
                sqg = scr.tile([P, 1024], dt.float32, tag="sqg")
                nc.gpsimd.tensor_tensor(sqg[:], xt[:, 3072:4096], xt[:, 3072:4096],
                                        op=ALU.mult)
                nc.vector.tensor_reduce(sqacc[:, 3:4], sqg[:], axis=AX.X, op=ALU.add)
                s1acc = stat.tile([P, 2], dt.float32, tag="s1acc")
                nc.vector.tensor_reduce(
                    s1acc[:, 0:1], xt[:, 0:2048], axis=AX.X, op=ALU.add
                )
                sxb = scr.tile([P, 2048], dt.float32, tag="sxb")
                nc.scalar.activation(
                    sxb[:], xt[:, 2048:4096], AF.Identity,
                    accum_out=s1acc[:, 1:2],
                )
                nc.vector.tensor_reduce(stats[:, 0:1], s1acc[:], axis=AX.X, op=ALU.add)
                nc.vector.tensor_reduce(stats[:, 1:2], sqacc[:], axis=AX.X, op=ALU.add)
                gps = gnps.tile([8, 2], dt.float32, tag="gps")
                nc.tensor.matmul(gps[:], gmat[:], stats[:], start=True, stop=True)
                gsb = stat.tile([8, 2], dt.float32, tag="gsb")
                nc.vector.tensor_copy(gsb[:], gps[:])
                cps = gnps.tile([P, 2], dt.float32, tag="cps")
                nc.tensor.matmul(cps[:], gmt[:], gsb[:], start=True, stop=True)
                cst = stat.tile([P, 2], dt.float32, tag="cst")
                nc.vector.tensor_copy(cst[:], cps[:])
                mean = stat.tile([P, 1], dt.float32, tag="mean")
                nc.vector.tensor_scalar_mul(mean[:], cst[:, 0:1], 1.0 / NPIX)
                msq = stat.tile([P, 1], dt.float32, tag="msq")
                nc.scalar.square(msq[:], mean[:])
                ex2 = stat.tile([P, 1], dt.float32, tag="ex2")
                nc.vector.tensor_scalar(
                    ex2[:], cst[:, 1:2], 1.0 / NPIX, EPS, op0=ALU.mult, op1=ALU.add
                )
                varp = stat.tile([P, 1], dt.float32, tag="varp")
                nc.vector.tensor_sub(varp[:], ex2[:], msq[:])
                rinv = stat.tile([P, 1], dt.float32, tag="rinv")
                nc.vector.reciprocal(rinv[:], varp[:])
                rstd = stat.tile([P, 1], dt.float32, tag="rstd")
                nc.scalar.sqrt(rstd[:], rinv[:])
                s_t = stat.tile([P, 1], dt.float32, tag="s_t")
                nc.vector.tensor_mul(s_t[:], rstd[:], bias["gnw"][t][:])
                ms = stat.tile([P, 1], dt.float32, tag="ms")
                nc.vector.tensor_mul(ms[:], mean[:], s_t[:])
                t_t = stat.tile([P, 1], dt.float32, tag="t_t")
                nc.vector.scalar_tensor_tensor(
                    t_t[:], ms[:], -1.0, bias["gnb"][t][:], op0=ALU.mult, op1=ALU.add
                )
                # h = x*s + t, split ACT/DVE/GpSimd
                dst = h2[t // 2][:, t % 2, :]
                nc.vector.tensor_scalar(
                    dst[0:P, 0:1536], xt[:, 0:1536],
                    s_t[:], t_t[:], op0=ALU.mult, op1=ALU.add,
                )
                nc.gpsimd.tensor_scalar(
                    dst[0:P, 1536:2560], xt[:, 1536:2560],
                    s_t[:], t_t[:], op0=ALU.mult, op1=ALU.add,
                )
                nc.scalar.activation(
                    dst[0:P, 2560:4096], xt[:, 2560:4096], AF.Identity,
                    bias=t_t[:], scale=s_t[:],
                )
        xpool_cm.__exit__(None, None, None)

        # weights (fp8, x16 from the host; loaded after GN emission so their
        # DMAs don't sit ahead of the GN reductions in queue order). wq/wk
        # gate the QKV start: put them on the SWDGE queues, which finish
        # their share of x earliest; wv/wp (needed later) go to the HW queues.
        w_engine = {"wq_t": nc.sync, "wk_t": nc.sync, "wv_t": nc.sync,
                    "wp_t": nc.sync}
        w2 = {}
        for n in ("wq_t", "wk_t", "wv_t", "wp_t"):
            pool = const if n == "wp_t" else ph1
            tiles = []
            for cc in range(CP):
                wb = pool.tile(
                    [P, 2, C], dt.float8e4, tag=f"{n}f8{cc}", name=f"{n}f8{cc}"
                )
                for h in range(2):
                    t = 2 * cc + h
                    w_engine[n].dma_start(wb[:, h, :], w_d[n][t * P : (t + 1) * P, :])
                tiles.append(wb)
            w2[n] = tiles
        wp2 = w2["wp_t"]

        # ------- q, k (fp8 pair tiles [128,2,HW]) and v_T ([128,2,C] x16) ----
        q2 = [
            qpool.tile([P, 2, HW], dt.float8e4, tag=f"q{c}", name=f"q{c}")
            for c in range(CP)
        ]
        k2 = [
            kpool.tile([P, 2, HW], dt.float8e4, tag=f"k{c}", name=f"k{c}")
            for c in range(CP)
        ]
        vt2 = [
            vtpool.tile([P, 2, C], dt.float8e4, tag=f"vt{m}", name=f"vt{m}")
            for m in range(MP)
        ]
        with tc.tile_pool(name="qkps", bufs=8, space="PSUM") as qkps:
            for wn, dst2, bn in (("wq_t", q2, "qb"), ("wk_t", k2, "kb")):
                for o in range(CT):
                    for j in range(NCH):
                        ps = qkps.tile([P, 512], dt.float32, tag="qkps")
                        for cc in range(CP):
                            nc.tensor.matmul(
                                ps[:],
                                w2[wn][cc][:, :, o * P : (o + 1) * P],
                                h2[cc][:, :, j * 512 : (j + 1) * 512],
                                start=(cc == 0),
                                stop=(cc == CP - 1),
                                perf_mode=PM.DoubleRow,
                            )
                        # evacuate (x 1/WSC, + bias), alternating DVE/ACT
                        d = dst2[o // 2][:, o % 2, j * 512 : (j + 1) * 512]
                        if j % 2 == 0:
                            nc.vector.tensor_scalar(
                                d, ps[:], 1.0 / WSC, bias[bn][o][:],
                                op0=ALU.mult, op1=ALU.add,
                            )
                        else:
                            nc.scalar.activation(
                                d, ps[:], AF.Identity,
                                bias=bias[bn][o][:], scale=1.0 / WSC,
                            )
            for m in range(MT):
                ps = qkps.tile([P, 512], dt.float32, tag="qkps")
                for cc in range(CP):
                    nc.tensor.matmul(
                        ps[:],
                        h2[cc][:, :, m * P : (m + 1) * P],
                        w2["wv_t"][cc][:],
                        start=(cc == 0),
                        stop=(cc == CP - 1),
                        perf_mode=PM.DoubleRow,
                    )
                d = vt2[m // 2][:, m % 2, :]
                if m % 2 == 0:
                    nc.vector.tensor_scalar_mul(d, ps[:], 1.0 / WSC)
                else:
                    nc.scalar.activation(d, ps[:], AF.Identity, scale=1.0 / WSC)
        phase1.__exit__(None, None, None)  # frees h + wq/wk/wv fp8 SBUF

        # ---------------- attention + proj, per n-chunk of 512 queries --------
        with (
            tc.tile_pool(name="spool", bufs=2, space="PSUM") as spool,
            tc.tile_pool(name="pvps", bufs=1, space="PSUM") as pvps,
            tc.tile_pool(name="ptpool", bufs=8) as ptpool,
            tc.tile_pool(name="ptacc", bufs=3) as ptaccp,
            tc.tile_pool(name="misc", bufs=2) as misc,
            tc.tile_pool(name="xres", bufs=4) as xres,
            tc.tile_pool(name="ystage", bufs=4) as ystage,
        ):
            def emit_proj(j, hu, linv_bc, op_list=(0, 1)):
                # y = P(hu) * (1/l) + pbp + x  -- division after the projection
                # (wp is x16 and hu is /16 in fp8, so the scales cancel here)
                for op in op_list:
                    pt_ps = spool.tile(
                        [P, 2, 512], dt.float32, tag="sp", name=f"pps{j}_{op}"
                    )
                    for oh in range(2):
                        o = 2 * op + oh
                        pps = pt_ps[:, oh, :]
                        for cc in range(CP):
                            nc.tensor.matmul(
                                pps,
                                wp2[cc][:, :, o * P : (o + 1) * P],
                                hu[cc][:],
                                start=(cc == 0),
                                stop=(cc == CP - 1),
                                perf_mode=PM.DoubleRow,
                            )
                        xr = xres.tile(
                            [P, 512], dt.float32, tag="xr", name=f"xr{j}_{o}"
                        )
                        nc.sync.dma_start(
                            xr[:], x_d[o * P : (o + 1) * P, j * 512 : (j + 1) * 512]
                        )
                        ppc = ystage.tile(
                            [P, 512], dt.float32, tag="ppc", name=f"ppc{j}_{o}"
                        )
                        nc.scalar.copy(ppc[:], pps)
                        pn = ystage.tile(
                            [P, 512], dt.float32, tag="pn", name=f"pn{j}_{o}"
                        )
                        yst = ystage.tile(
                            [P, 512], dt.float32, tag="yst", name=f"y{j}_{o}"
                        )
                        halves = (slice(0, 256), slice(256, 512)) if j == NCH - 1 \
                            else (slice(0, 512),)
                        for hs in halves:
                            nc.vector.tensor_mul(pn[:, hs], ppc[:, hs],
                                                 linv_bc[:, hs])
                            nc.vector.scalar_tensor_tensor(
                                yst[:, hs], pn[:, hs], bias["pbp"][o][:], xr[:, hs],
                                op0=ALU.add, op1=ALU.add,
                            )
                        nc.sync.dma_start(
                            y_d[o * P : (o + 1) * P, j * 512 : (j + 1) * 512], yst[:]
                        )

            def emit_norm_a(j, accs, pt_last, split=False):
                # ones-matmuls merge both partial exp sums (pairs 0..14) plus
                # the final exp pair straight from fp8 (so the norm does not
                # chase the accumulator engines at the chunk seam): out[m, n]
                # = l[n] on every row -- broadcast for free; then reciprocal.
                lt = spool.tile([P, 2, 512], dt.float32, tag="sp", name=f"lf{j}")
                nc.tensor.matmul(lt[:, 0, :], ones_col[:], accs[0][:],
                                 start=True, stop=False)
                nc.tensor.matmul(lt[:, 0, :], ones_col[:], accs[1][:],
                                 start=False, stop=False)
                nc.tensor.matmul(lt[:, 0, :], ones_pair[:], pt_last[:],
                                 start=False, stop=True, perf_mode=PM.DoubleRow)
                # copy l out of PSUM first: the slow reciprocal must not pin
                # the PSUM ring slot (the next scores alloc waits on it)
                lcp = misc.tile([P, 512], dt.float32, tag="lcp", name=f"lcp{j}")
                nc.scalar.copy(lcp[:], lt[:, 0, :])
                linv_bc = misc.tile([P, 512], dt.float32, tag="linvbc", name=f"lbc{j}")
                if split:
                    # tail: halve the serial reciprocal so the first proj pn
                    # can start ~1.6us earlier
                    nc.vector.reciprocal(linv_bc[:, 0:256], lcp[:, 0:256])
                    nc.vector.reciprocal(linv_bc[:, 256:512], lcp[:, 256:512])
                else:
                    nc.vector.reciprocal(linv_bc[:], lcp[:])
                return linv_bc

            # flat software pipeline over all (chunk, key-pair) steps: the PV
            # matmuls trail the score/exp stream by a global lag of 2 pairs,
            # flowing across chunk boundaries so the PE never sees a seam.
            PVLAG = 4
            state = {}   # j -> dict(pv_ps, accs, pts)
            hu_by = {}   # j -> hu pair tiles
            linv_by = {}

            def emit_scores(j, ii):
                st = state[j]
                s_ps = spool.tile([P, 2, 512], dt.float32, tag="sp", name=f"s{j}_{ii}")
                for h in range(2):
                    i = 2 * ii + h
                    for cc in range(CP):
                        nc.tensor.matmul(
                            s_ps[:, h, :],
                            k2[cc][:, :, i * P : (i + 1) * P],
                            q2[cc][:, :, j * 512 : (j + 1) * 512],
                            start=(cc == 0),
                            stop=(cc == CP - 1),
                            perf_mode=PM.DoubleRow,
                        )
                pt = ptpool.tile([P, 2, 512], dt.float8e4, tag="pt",
                                 name=f"pt{j}_{ii}")
                # fused exp over both PSUM banks, with the -2 logit shift
                nc.scalar.activation(pt[:], s_ps[:], AF.Exp, bias=nbias[:],
                                     scale=SCALE)
                st["pts"][ii] = pt
                accA, accB = st["accs"]
                if ii == 0:
                    nc.vector.tensor_copy(accA[:], pt[:, 0, :])
                    nc.gpsimd.tensor_copy(accB[:], pt[:, 1, :])
                elif ii < MP - 1:  # last pair is summed by the norm matmul
                    nc.vector.tensor_add(accA[:], accA[:], pt[:, 0, :])
                    nc.gpsimd.tensor_add(accB[:], accB[:], pt[:, 1, :])

            def emit_pv(j, ii):
                st = state[j]
                for c in range(CT):
                    nc.tensor.matmul(
                        st["pv_ps"][c][:],
                        vt2[ii][:, :, c * P : (c + 1) * P],
                        st["pts"][ii][:],
                        start=(ii == 0),
                        stop=(ii == MP - 1),
                        perf_mode=PM.DoubleRow,
                    )

            def emit_hu_evac(j):
                # unnormalized PV (/16) to fp8 SBUF frees the banks for the
                # next chunk; split DVE/ACT so neither stream hiccups
                hu = [
                    misc.tile([P, 2, 512], dt.float8e4, tag=f"hu{cc}",
                              name=f"hu{j}_{cc}")
                    for cc in range(CP)
                ]
                pv_ps = state[j]["pv_ps"]
                for c in range(CT):
                    d = hu[c // 2][:, c % 2, :]
                    if c < 2 and j < NCH - 1:
                        nc.vector.tensor_scalar_mul(d, pv_ps[c][:], 1.0 / WSC)
                    else:
                        nc.scalar.activation(d, pv_ps[c][:], AF.Identity,
                                             scale=1.0 / WSC)
                hu_by[j] = hu

            NSTEP = NCH * MP
            for g in range(NSTEP + PVLAG):
                j, ii = divmod(g, MP)
                if ii == 0 and j < NCH:
                    state[j] = {
                        "pv_ps": [
                            pvps.tile([P, 512], dt.float32, tag=f"pv{c}",
                                      name=f"pv{j}_{c}")
                            for c in range(CT)
                        ],
                        # two independent running exp sums (DVE + GpSimd),
                        # merged by the two-step ones-matmul in emit_norm_a
                        "accs": (
                            ptaccp.tile([P, 512], dt.bfloat16, tag="accA",
                                        name=f"accA{j}"),
                            ptaccp.tile([P, 512], dt.bfloat16, tag="accB",
                                        name=f"accB{j}"),
                        ),
                        "pts": [None] * MP,
                    }
                if g < NSTEP:
                    emit_scores(j, ii)
                gp = g - PVLAG
                if gp >= 0:
                    jp, iip = divmod(gp, MP)
                    emit_pv(jp, iip)
                    if iip == MP - 1:
                        emit_hu_evac(jp)
                # per-chunk norm/proj of the previous chunk, off the seam;
                # the last chunk's norm fires immediately after its final exp
                # so the tail reciprocal overlaps the PV drain
                if ii == 1 and 1 <= j < NCH:
                    linv_by[j - 1] = emit_norm_a(
                        j - 1, state[j - 1]["accs"], state[j - 1]["pts"][MP - 1]
                    )
                if ii == 4 and 1 <= j < NCH:
                    emit_proj(j - 1, hu_by[j - 1], linv_by[j - 1], op_list=(0,))
                if ii == 6 and 1 <= j < NCH:
                    emit_proj(j - 1, hu_by[j - 1], linv_by[j - 1], op_list=(1,))
                if j == NCH - 1 and ii == MP - 1:
                    linv_by[j] = emit_norm_a(j, state[j]["accs"],
                                             state[j]["pts"][MP - 1], split=True)

            emit_proj(NCH - 1, hu_by[NCH - 1], linv_by[NCH - 1])

    _legalize_waits(nc)
    return nc


def _get_nc():
    if "nc" not in _CACHE:
        _CACHE["nc"] = _build()
    return _CACHE["nc"]


def _in_maps(x, gn_w, gn_b, q_w, q_b, k_w, k_b, v_w, v_b, proj_w, proj_b):
    x = np.ascontiguousarray(np.asarray(x, dtype=np.float32))
    assert x.shape == (B, C, 64, 64)
    f32 = np.float32
    gmat = np.zeros((P, 8), f32)
    gmat[np.arange(P), np.arange(P) // 16] = 1.0
    # pbp = proj_b + Wp @ v_b (division-after-projection folds v's bias into a
    # constant output bias); computed on host in f64.
    pbp = (
        np.asarray(proj_b, np.float64)
        + np.asarray(proj_w, np.float64) @ np.asarray(v_b, np.float64)
    ).astype(f32)
    bpack = np.stack(
        [
            np.asarray(a, f32).reshape(C)
            for a in (q_b, k_b, pbp, gn_w, gn_b)
        ],
        axis=1,
    )  # [C, 5]
    import ml_dtypes

    bf16 = ml_dtypes.bfloat16
    f8 = ml_dtypes.float8_e4m3

    def wprep(w):
        return np.ascontiguousarray(
            (np.asarray(w, f32).T * np.float32(WSC)).astype(f8)
        )

    shared = {
        "wq_t": wprep(q_w),
        "wk_t": wprep(k_w),
        "wv_t": wprep(v_w),
        "wp_t": wprep(proj_w),
        "bpack": np.ascontiguousarray(bpack),
        "gmat": gmat,
        "gmat_t": np.ascontiguousarray(gmat.T),
    }

    return [
        dict(
            shared,
            x=x[b].reshape(C, HW),
            x_bf=x[b].reshape(C, HW).astype(f8),
        )
        for b in range(B)
    ]


def kernel(x, gn_w, gn_b, q_w, q_b, k_w, k_b, v_w, v_b, proj_w, proj_b):
    in_maps = _in_maps(x, gn_w, gn_b, q_w, q_b, k_w, k_b, v_w, v_b, proj_w, proj_b)
    nc = _get_nc()
    res = run_bass_kernel_spmd(nc, in_maps, core_ids=list(range(B)))
    out = np.stack([res.results[b]["y"].reshape(C, 64, 64) for b in range(B)])
    return out.astype(np.float32)


def run_traced(x, gn_w, gn_b, q_w, q_b, k_w, k_b, v_w, v_b, proj_w, proj_b):
    """Like kernel() but with NTFF profiling; returns (out, results)."""
    _install_ntff_hook()
    in_maps = _in_maps(x, gn_w, gn_b, q_w, q_b, k_w, k_b, v_w, v_b, proj_w, proj_b)
    nc = _get_nc()
    res = run_bass_kernel_spmd(nc, in_maps, core_ids=list(range(B)), trace=True)
    out = np.stack([res.results[b]["y"].reshape(C, 64, 64) for b in range(B)])
    return out.astype(np.float32), res


def _install_ntff_hook():
    if "antenv.axon_hooks" in sys.modules:
        return
    sys.path.insert(0, "/root/.axon_site")
    try:
        from trn_agent_boot.trn_boot import _ntff_profile_via_ctypes

        hook = _ntff_profile_via_ctypes("/opt/axon/libaxon_pjrt.so")
    except Exception:
        hook = None
    mod = types.ModuleType("antenv.axon_hooks")
    mod.get_axon_ntff_profile_hook = lambda: hook
    sys.modules["antenv.axon_hooks"] = mod
